# revision 29
# baseline (speedup 1.0000x reference)
"""GQA attention kernel for Trainium2, SPMD across 8 NeuronCores.

Fast path (causal mask, detected on host):
  core = (batch b, lane w).  Per batch, the 16 q-blocks of 128 rows are
  dealt to lanes in extent tiers: lane w owns blocks {15-w, 11-w, 7-w, 3-w},
  processed in 4 "slots" with uniform padded k-extents (16, 12, 8, 4)
  k-tiles of 128 keys.  Every core runs the identical program (SPMD
  requirement); causality makes the padded work exp(-inf)=0 via a hosted
  mask add on the last 128 q-columns of each k-tile.

  K/V projections are computed seq-sharded (each lane ropes/projects its
  own 512-seq chunk) and AllGathered across the 4 lanes of a batch through
  DRAM bounce buffers; the Q projection overlaps the collective.

  All weights/activations are packed on the host so that every DMA moves
  multi-KB contiguous lines per partition (one DMA per projection head /
  kv group / output column block) -- small-line weight DMAs were the
  dominant stall in earlier revisions.

  All matmuls run in bf16 with fp32 PSUM accumulation, feature-major
  layouts so every matmul contracts over the partition dim:
    scores^T[k,q] = (K^T tile).T @ Q^T tile     (k-major scores)
    AV^T[d,q]    = (V tile).T @ exp^T tile      (V kept seq-major)
    out[q,o]     = (AV^T tile).T @ Wo^T tile
  softmax sum over k: exp tiles are pair/quad-summed on the vector engine
  and a single ones-matmul per tier folds them into PSUM (instead of one
  ones-matmul per k-tile); the AV matmuls lag the score matmuls by two
  k-tiles so the PE never waits on the scalar-engine exp.

Fallback path (any non-causal mask): dense attention, data-parallel over
batch x query-window, K/V recomputed per core (previous generation kernel).
"""

import numpy as np
from ml_dtypes import bfloat16

B, S, H = 2, 2048, 2304
NH, NKV, HD = 9, 3, 256
GROUPS = NH // NKV
ROPE_BASE = 100000.0
SQ = 512            # query rows per core
NCORES = 8
P = 128
NHC = H // P        # 18 H-chunks
DK = NKV * HD       # 768
NKT = S // P        # 16 k-tiles
CHUNK = S // 4      # 512 seq rows per lane for K/V projection
NQ = SQ // P        # 4
SCALE = 1.0 / 16.0  # 1/sqrt(HD)

EXT = (16, 12, 8, 4)  # padded k-extent (in 128-key tiles) per slot


def _qblocks(w):
    """128-row q-block index (0..15) owned by lane w, per slot."""
    return [15 - 4 * j - w for j in range(4)]


_CACHE = {}


def _rope_tables():
    inv_freq = 1.0 / (ROPE_BASE ** (np.arange(0, HD, 2, dtype=np.float32) / HD))
    t = np.arange(S, dtype=np.float32)
    freqs = np.outer(t, inv_freq).astype(np.float32)      # [S, 128]
    cos = np.cos(freqs).T                                  # [128, S]
    sin = np.sin(freqs).T
    return cos, sin


def _is_causal(mask):
    q = np.arange(S)[:, None]
    k = np.arange(S)[None, :]
    tril = k <= q
    return bool(np.all(mask[tril] == 0.0) and np.all(mask[~tril] <= -1e8))


# ---------------------------------------------------------------------------
# fast causal kernel
# ---------------------------------------------------------------------------

def _build_nc_fast():
    import concourse.bass as bass
    import concourse.tile as tile
    from concourse import bacc, mybir
    from concourse.mybir import ActivationFunctionType

    BF = mybir.dt.bfloat16
    F32 = mybir.dt.float32

    nc = bacc.Bacc(None, target_bir_lowering=False, debug=False,
                   num_devices=NCORES)

    # activations packed [128, 18*512]: partition line = all 18 h-chunks
    d_xkv = nc.dram_tensor("xkv", [P, NHC * CHUNK], BF, kind="ExternalInput").ap()
    d_xq = nc.dram_tensor("xq", [P, NHC * SQ], BF, kind="ExternalInput").ap()
    # weights packed per consumer unit as 9KB-per-partition slabs
    # [128, 18*256] so one DMA covers a whole head / K-or-V group / og half
    SLAB = NHC * HD          # 4608 cols = 9KB bf16 per partition
    d_wqp = nc.dram_tensor("wqp", [NH * P * SLAB], BF,
                           kind="ExternalInput").ap()
    d_wkvp = nc.dram_tensor("wkvp", [2 * NKV * P * SLAB], BF,
                            kind="ExternalInput").ap()
    d_wop = nc.dram_tensor("wop", [9 * P * SLAB], BF,
                           kind="ExternalInput").ap()

    d_cosk = nc.dram_tensor("cosk", [P, CHUNK], BF, kind="ExternalInput").ap()
    d_sink = nc.dram_tensor("sink", [P, CHUNK], BF, kind="ExternalInput").ap()
    d_cosq = nc.dram_tensor("cosq", [P, SQ], BF, kind="ExternalInput").ap()
    d_sinq = nc.dram_tensor("sinq", [P, SQ], BF, kind="ExternalInput").ap()
    # pre-interleaved scaled mask: maskp[p, t*128+qi] = 16*mask[qrow(slot_t, qi), t*128+p]
    d_maskp = nc.dram_tensor("maskp", [P, NKT * P], BF, kind="ExternalInput").ap()
    d_out = nc.dram_tensor("out", [SQ, H], BF, kind="ExternalOutput").ap()
    if _CACHE.get("debug"):
        d_dbg_avt = nc.dram_tensor("dbg_avt", [P, NHC * SQ], BF,
                                   kind="ExternalOutput").ap()
        d_dbg_kt = nc.dram_tensor("dbg_kt", [P, 2 * NKT * P], BF,
                                  kind="ExternalOutput").ap()
        d_dbg_v = nc.dram_tensor("dbg_v", [P, NKT * HD], BF,
                                 kind="ExternalOutput").ap()
        d_dbg_qt = nc.dram_tensor("dbg_qt", [P, 2 * SQ], BF,
                                  kind="ExternalOutput").ap()
        d_dbg_sum = nc.dram_tensor("dbg_sum", [P, NH * SQ], mybir.dt.float32,
                                   kind="ExternalOutput").ap()

    def slab_view(d, idx):
        base = idx * P * SLAB
        return d[base:base + P * SLAB].rearrange("(p c) -> p c", p=P)

    with tile.TileContext(nc) as tc:
        with (
            tc.tile_pool(name="res", bufs=1) as res,
            tc.tile_pool(name="wt", bufs=4) as wt_pool,
            tc.tile_pool(name="rtmp", bufs=6) as rtmp_pool,
            tc.tile_pool(name="kvout", bufs=8) as kvout_pool,
            tc.tile_pool(name="expt", bufs=6) as expt_pool,
            tc.tile_pool(name="esum", bufs=4) as esum_pool,
            tc.tile_pool(name="recip", bufs=3) as recip_pool,
            tc.tile_pool(name="osb", bufs=4) as osb_pool,
            tc.tile_pool(name="ps", bufs=8, space="PSUM") as ps_pool,
            tc.tile_pool(name="dram", bufs=1, space="DRAM") as dram_pool,
        ):
            # ---- resident tiles ----
            ones_sb = res.tile([P, P], BF, tag="ones")
            nc.vector.memset(ones_sb[:], 1.0)

            # PE warmup: release the HAM throttle while the first DMAs land
            junk_ps = ps_pool.tile([P, SQ], F32, tag="ps", name="junk")
            for _ in range(224):
                nc.tensor.matmul(junk_ps[:, :P], ones_sb[:], ones_sb[:],
                                 start=True, stop=True)

            # CC ring warmup: a tiny dummy AllGather absorbs the first-
            # collective setup latency so the real gathers fire promptly
            ccw_in = dram_pool.tile([1, P], BF, name="ccwi")
            ccw_out = dram_pool.tile([4, P], BF, name="ccwo")
            nc.gpsimd.collective_compute(
                "AllGather", mybir.AluOpType.bypass,
                replica_groups=[[0, 1, 2, 3], [4, 5, 6, 7]],
                ins=[ccw_in[:]], outs=[ccw_out[:]])

            cosk_sb = res.tile([P, CHUNK], BF, tag="cosk")
            nc.scalar.dma_start(cosk_sb[:], d_cosk[:])
            sink_sb = res.tile([P, CHUNK], BF, tag="sink")
            nc.scalar.dma_start(sink_sb[:], d_sink[:])

            # packed X^T for this lane's K/V chunk.  Half-DMAs interleaved
            # with the first weight slabs so sweep 0 starts after ~2.4MB
            # lands (subtile deps cover the first 9 h-chunks).
            HC = NHC * CHUNK // 2
            xkv_sb = res.tile([P, NHC * CHUNK], BF, tag="xkv")
            wkv0 = [wt_pool.tile([P, SLAB], BF, tag="wt", name=f"wkv0_{i}")
                    for i in range(2)]
            nc.sync.dma_start(xkv_sb[:, :HC], d_xkv[:, :HC])
            nc.sync.dma_start(wkv0[0][:, :SLAB // 2],
                              slab_view(d_wkvp, 0)[:, :SLAB // 2])
            nc.sync.dma_start(wkv0[1][:, :SLAB // 2],
                              slab_view(d_wkvp, 1)[:, :SLAB // 2])
            nc.sync.dma_start(xkv_sb[:, HC:], d_xkv[:, HC:])
            nc.sync.dma_start(wkv0[0][:, SLAB // 2:],
                              slab_view(d_wkvp, 0)[:, SLAB // 2:])
            nc.sync.dma_start(wkv0[1][:, SLAB // 2:],
                              slab_view(d_wkvp, 1)[:, SLAB // 2:])
            # prefetch sweep 1's weights from t=0: the two remaining slabs
            # are untouched, so these transfers queue right behind sweep 0's
            wkv1 = [wt_pool.tile([P, SLAB], BF, tag="wt", name=f"wkv1_{i}")
                    for i in range(2)]
            nc.sync.dma_start(wkv1[0][:], slab_view(d_wkvp, 2))
            nc.sync.dma_start(wkv1[1][:], slab_view(d_wkvp, 3))

            # rope'd Q^T, one tile per head ([dq-half0 | dq-half1])
            qt_h = [res.tile([P, 2 * SQ], BF, tag=f"qt{hh}", name=f"qth{hh}")
                    for hh in range(NH)]
            # gathered K^T per kv head: col block (m, t) at (m*NKT + t)*P
            kt_g = [res.tile([P, 2 * NKT * P], BF, tag=f"kt{g}",
                          name=f"ktg{g}") for g in range(NKV)]
            # gathered V per kv head: col block t at t*HD (+m*P for half m)
            v_g = [res.tile([P, NKT * HD], BF, tag=f"v{g}",
                         name=f"vg{g}") for g in range(NKV)]
            avt_sb = res.tile([P, NHC * SQ], BF, tag="avt")    # AV^T

            # one DRAM bounce buffer for all 3 kv heads, exchanged in a
            # single AllGather (one ring setup instead of three): group g's
            # K_g^T [HD, CHUNK] at rows g*CCR, then its V region.
            CCR = HD + CHUNK // 2          # rows per group block (512)
            cc_in = dram_pool.tile([NKV * CCR, CHUNK], BF, name="cci")
            cc_out = dram_pool.tile([4 * NKV * CCR, CHUNK], BF, name="cco")

            # V region of a group block viewed [128, 4*HD]: partition p's
            # line = concat over sub-blocks sb of V[sb*128+p, :].  Gathered
            # loads then read one contiguous 2KB run per partition.
            def vin_view(g):
                f = cc_in[:].flatten()
                base = (g * CCR + HD) * CHUNK
                return f[base:base + CHUNK // 2 * CHUNK].rearrange(
                    "(p x) -> p x", p=P)

            def vout_view(g, r):
                f = cc_out[:].flatten()
                base = ((r * NKV + g) * CCR + HD) * CHUNK
                return f[base:base + CHUNK // 2 * CHUNK].rearrange(
                    "(p x) -> p x", p=P)

            def rope_pair(top_ps, bot_ps, cos_sb, sin_sb, width,
                          out_ap_top, out_ap_bot):
                # out_top = top*cos - bot*sin ; out_bot = bot*cos + top*sin
                ta = rtmp_pool.tile([P, SQ], F32, tag="rt")
                nc.vector.tensor_mul(ta[:, :width], top_ps, cos_sb[:, :width])
                tb = rtmp_pool.tile([P, SQ], F32, tag="rt")
                nc.vector.tensor_mul(tb[:, :width], bot_ps, sin_sb[:, :width])
                nc.vector.tensor_sub(out_ap_top, ta[:, :width], tb[:, :width])
                tc_ = rtmp_pool.tile([P, SQ], F32, tag="rt")
                nc.vector.tensor_mul(tc_[:, :width], bot_ps, cos_sb[:, :width])
                td = rtmp_pool.tile([P, SQ], F32, tag="rt")
                nc.vector.tensor_mul(td[:, :width], top_ps, sin_sb[:, :width])
                nc.vector.tensor_add(out_ap_bot, tc_[:, :width], td[:, :width])

            # ---- K+V projection per kv head g; AllGather K_g||V_g eagerly,
            # interleaved with Q-projection head groups so the PE never
            # starves while the collectives run ----
            groups = [[0, 1, 2, 3], [4, 5, 6, 7]]

            def emit_kv_sweep(g):
                if g == 0:
                    wk_t, wv_t = wkv0
                elif g == 1:
                    wk_t, wv_t = wkv1
                else:
                    wk_t = wt_pool.tile([P, SLAB], BF, tag="wt", name=f"wk{g}")
                    nc.sync.dma_start(wk_t[:], slab_view(d_wkvp, 2 * g))
                    wv_t = wt_pool.tile([P, SLAB], BF, tag="wt", name=f"wv{g}")
                    nc.sync.dma_start(wv_t[:], slab_view(d_wkvp, 2 * g + 1))
                ka = [ps_pool.tile([P, CHUNK], F32, tag="ps", name="kacc")
                      for _ in range(2)]
                va = [ps_pool.tile([P, HD], F32, tag="ps", name="vacc")
                      for _ in range(4)]
                for h in range(NHC):
                    for m in range(2):
                        nc.tensor.matmul(
                            ka[m][:],
                            wk_t[:, h * HD + m * P:h * HD + (m + 1) * P],
                            xkv_sb[:, h * CHUNK:(h + 1) * CHUNK],
                            start=(h == 0), stop=(h == NHC - 1))
                    for sb in range(4):
                        nc.tensor.matmul(
                            va[sb][:],
                            xkv_sb[:, h * CHUNK + sb * P:h * CHUNK + (sb + 1) * P],
                            wv_t[:, h * HD:(h + 1) * HD],
                            start=(h == 0), stop=(h == NHC - 1))
                ktop = kvout_pool.tile([P, CHUNK], BF, tag="kvo")
                kbot = kvout_pool.tile([P, CHUNK], BF, tag="kvo")
                rope_pair(ka[0][:], ka[1][:], cosk_sb, sink_sb, CHUNK,
                          ktop[:], kbot[:])
                # bounce writes on the scalar queue: the sync queue's weight
                # backlog would delay the collective's doorbell
                nc.scalar.dma_start(cc_in[g * CCR:g * CCR + P, :], ktop[:])
                nc.scalar.dma_start(cc_in[g * CCR + P:g * CCR + 2 * P, :],
                                    kbot[:])
                vview = vin_view(g)
                for sb in range(4):
                    vt = kvout_pool.tile([P, HD], BF, tag="kvo")
                    nc.vector.tensor_copy(vt[:], va[sb][:])
                    nc.scalar.dma_start(vview[:, sb * HD:(sb + 1) * HD],
                                        vt[:])

            def emit_gather():
                nc.gpsimd.collective_compute(
                    "AllGather", mybir.AluOpType.bypass, replica_groups=groups,
                    ins=[cc_in[:]], outs=[cc_out[:]])
                for g in range(NKV):
                    for r in range(4):
                        base = (r * NKV + g) * CCR
                        for m in range(2):
                            nc.gpsimd.dma_start(
                                kt_g[g][:, (m * NKT + 4 * r) * P:
                                        (m * NKT + 4 * r + 4) * P],
                                cc_out[base + m * P:base + (m + 1) * P, :])
                        nc.gpsimd.dma_start(
                            v_g[g][:, 4 * r * HD:(4 * r + 4) * HD],
                            vout_view(g, r))

            def emit_q_head(hh):
                # weight slab split across the scalar and sync DMA queues:
                # neither alone sustains the PE's ~125GB/s weight demand
                wq_t = wt_pool.tile([P, SLAB], BF, tag="wt", name=f"wq{hh}")
                nc.scalar.dma_start(wq_t[:, :SLAB // 2],
                                    slab_view(d_wqp, hh)[:, :SLAB // 2])
                nc.sync.dma_start(wq_t[:, SLAB // 2:],
                                  slab_view(d_wqp, hh)[:, SLAB // 2:])
                accs = [ps_pool.tile([P, SQ], F32, tag="ps", name="qacc")
                        for _ in range(2)]
                for h in range(NHC):
                    for j in range(2):
                        nc.tensor.matmul(
                            accs[j][:],
                            wq_t[:, h * HD + j * P:h * HD + (j + 1) * P],
                            xq_sb[:, h * SQ:(h + 1) * SQ],
                            start=(h == 0), stop=(h == NHC - 1))
                rope_pair(accs[0][:], accs[1][:],
                          cosq_sb, sinq_sb, SQ,
                          qt_h[hh][:, 0:SQ], qt_h[hh][:, SQ:2 * SQ])

            cosq_sb = res.tile([P, SQ], BF, tag="cosq")
            nc.scalar.dma_start(cosq_sb[:], d_cosq[:])
            sinq_sb = res.tile([P, SQ], BF, tag="sinq")
            nc.scalar.dma_start(sinq_sb[:], d_sinq[:])
            xq_sb = res.tile([P, NHC * SQ], BF, tag="xq")
            nc.scalar.dma_start(xq_sb[:], d_xq[:])
            maskp_sb = res.tile([P, NKT * P], BF, tag="maskp")
            nc.scalar.dma_start(maskp_sb[:], d_maskp[:])

            emit_kv_sweep(0)
            emit_kv_sweep(1)
            emit_kv_sweep(2)
            emit_gather()
            for hh in range(NH):
                emit_q_head(hh)

            # ---- attention per q-head ----
            # scores for k-tile t retire through exp on the scalar engine;
            # the AV matmuls lag two k-tiles so the PE never waits on exp.
            # softmax denominator: e-tiles are pair/quad summed on the DVE
            # and folded into sum_ps by one ones-matmul per 4-tile tier.
            def emit_attn_head(hh):
                g = hh // GROUPS
                qtop = qt_h[hh][:, 0:SQ]
                qbot = qt_h[hh][:, SQ:2 * SQ]
                e_tiles = [None] * NKT
                es01 = [None] * 4
                esq = [None] * 4
                av_ps = None
                sum_ps = None

                for t in range(NKT + 2):
                    if t < NKT:
                        W = (4 - t // 4) * P
                        s_ps = ps_pool.tile([P, SQ], F32, tag="ps", name="sps")
                        nc.tensor.matmul(
                            s_ps[:, :W],
                            kt_g[g][:, t * P:(t + 1) * P],
                            qtop[:, :W], start=True, stop=False)
                        nc.tensor.matmul(
                            s_ps[:, :W],
                            kt_g[g][:, (NKT + t) * P:(NKT + t + 1) * P],
                            qbot[:, :W], start=False, stop=True)
                        # mask add on the last 128 active q-columns
                        nc.vector.tensor_add(s_ps[:, W - P:W], s_ps[:, W - P:W],
                                             maskp_sb[:, t * P:(t + 1) * P])
                        e_t = expt_pool.tile([P, SQ], BF, tag="et")
                        nc.scalar.activation(e_t[:, :W], s_ps[:, :W],
                                             ActivationFunctionType.Exp,
                                             scale=SCALE)
                        e_tiles[t] = e_t
                        T, j = t // 4, t % 4
                        if j == 1:
                            es = esum_pool.tile([P, SQ], BF, tag="es")
                            nc.vector.tensor_add(es[:, :W],
                                                 e_tiles[t - 1][:, :W],
                                                 e_t[:, :W])
                            es01[T] = es
                        elif j == 3:
                            es23 = esum_pool.tile([P, SQ], BF, tag="es")
                            nc.vector.tensor_add(es23[:, :W],
                                                 e_tiles[t - 1][:, :W],
                                                 e_t[:, :W])
                            esT = esum_pool.tile([P, SQ], BF, tag="es")
                            nc.vector.tensor_add(esT[:, :W], es01[T][:, :W],
                                                 es23[:, :W])
                            esq[T] = esT
                    if t == 1:
                        av_ps = [ps_pool.tile([P, SQ], F32, tag="ps",
                                              name="avps") for _ in range(2)]
                        sum_ps = ps_pool.tile([P, SQ], F32, tag="ps",
                                              name="sumps")
                    tt = t - 2
                    if tt < 0:
                        continue
                    W = (4 - tt // 4) * P
                    e_t = e_tiles[tt]
                    vsl = [v_g[g][:, tt * HD + m * P:tt * HD + (m + 1) * P]
                           for m in range(2)]
                    if tt % 4 == 3 and tt != NKT - 1:
                        # tier boundary: columns [W-P, W) retire here
                        for m in range(2):
                            nc.tensor.matmul(av_ps[m][:, :W - P], vsl[m],
                                             e_t[:, :W - P],
                                             start=False, stop=False)
                            nc.tensor.matmul(av_ps[m][:, W - P:W], vsl[m],
                                             e_t[:, W - P:W],
                                             start=False, stop=True)
                    else:
                        for m in range(2):
                            nc.tensor.matmul(av_ps[m][:, :W], vsl[m],
                                             e_t[:, :W],
                                             start=(tt == 0),
                                             stop=(tt == NKT - 1))
                    if tt % 4 == 0 and tt > 0:
                        # fold tier T's quad-summed exps into sum_ps, one
                        # tile after the tier's last av so the DVE quad sum
                        # stays off the PE critical path.  One instruction
                        # per tier: a second start=True into the same bank
                        # would re-zero it, and stop has no hardware effect,
                        # so retired columns just keep their values.
                        T = tt // 4 - 1
                        WT = (4 - T) * P
                        nc.tensor.matmul(sum_ps[:, :WT], ones_sb[:],
                                         esq[T][:, :WT],
                                         start=(T == 0), stop=False,
                                         skip_group_check=True)

                nc.tensor.matmul(sum_ps[:, :P], ones_sb[:], esq[3][:, :P],
                                 start=False, stop=True,
                                 skip_group_check=True)
                if _CACHE.get("debug"):
                    sum_sb = recip_pool.tile([P, SQ], F32, tag="rc")
                    nc.vector.tensor_copy(sum_sb[:], sum_ps[:])
                    nc.sync.dma_start(
                        d_dbg_sum[:, hh * SQ:(hh + 1) * SQ], sum_sb[:])
                rec = recip_pool.tile([P, SQ], F32, tag="rc")
                nc.vector.reciprocal_approx_fast(rec[:], sum_ps[:])
                for m in range(2):
                    nc.vector.tensor_mul(
                        avt_sb[:, (2 * hh + m) * SQ:(2 * hh + m + 1) * SQ],
                        av_ps[m][:], rec[:])

            for hh in range(NH):
                emit_attn_head(hh)

            if _CACHE.get("debug"):
                nc.sync.dma_start(d_dbg_avt[:], avt_sb[:])
                nc.sync.dma_start(d_dbg_kt[:], kt_g[0][:])
                nc.sync.dma_start(d_dbg_v[:], v_g[0][:])
                nc.sync.dma_start(d_dbg_qt[:], qt_h[0][:])

            # ---- output projection: out[q, o] = AV^T.T @ Wo^T ----
            # 5 column units (4x512 + 1x256); each unit's weights arrive as
            # one or two 9KB slabs.  The 256-col halves share the avt
            # stationary so LDWEIGHTS amortizes over 512 moving columns.
            slab_idx = 0
            for og, ow in ((0, 512), (512, 512), (1024, 512), (1536, 512),
                           (2048, 256)):
                halves = ow // HD
                wts = []
                for j in range(halves):
                    wt = wt_pool.tile([P, SLAB], BF, tag="wt",
                                      name=f"wo{slab_idx}")
                    nc.sync.dma_start(wt[:], slab_view(d_wop, slab_idx))
                    wts.append(wt)
                    slab_idx += 1
                oaccs = [ps_pool.tile([P, SQ], F32, tag="ps", name="oacc")
                         for _ in range(NQ)]
                # m-major so each q-block's accumulation finishes early and
                # its copy-out overlaps the remaining matmuls
                for m in range(NQ):
                    for c in range(NHC):
                        for j in range(halves):
                            # start=True zeroes the whole 2KB PSUM bank, so
                            # only the first write may carry it; the j=1
                            # half accumulates onto the freshly zeroed bank.
                            nc.tensor.matmul(
                                oaccs[m][:, j * HD:(j + 1) * HD],
                                avt_sb[:, c * SQ + m * P:c * SQ + (m + 1) * P],
                                wts[j][:, c * HD:(c + 1) * HD],
                                start=(c == 0 and j == 0),
                                stop=(c == NHC - 1),
                                skip_group_check=True)
                    o_sb = osb_pool.tile([P, SQ], BF, tag="ob")
                    nc.scalar.activation(o_sb[:, :ow], oaccs[m][:, :ow],
                                         ActivationFunctionType.Copy)
                    nc.sync.dma_start(d_out[m * P:(m + 1) * P, og:og + ow],
                                      o_sb[:, :ow])

    nc.compile()
    return nc


def _pack_cols(mat, col_blocks):
    """Pack [2304, w] column blocks of `mat` into [128, 18*Σw]:
    partition p's line = concat over blocks of (chunk h rows h*128+p)."""
    parts = []
    for c0, w in col_blocks:
        blk = np.ascontiguousarray(mat[:, c0:c0 + w])       # [2304, w]
        parts.append(blk.reshape(NHC, P, w))
    # [NHC, P, sum_w] -> [P, NHC, sum_w]
    cat = np.concatenate(parts, axis=2) if len(parts) > 1 else parts[0]
    return np.ascontiguousarray(cat.transpose(1, 0, 2)).reshape(P, -1)


def _pack_act(xt_cols):
    """[2304, w] -> [128, 18*w] with partition line = h-chunk-major."""
    w = xt_cols.shape[1]
    return np.ascontiguousarray(
        xt_cols.reshape(NHC, P, w).transpose(1, 0, 2)).reshape(P, -1)


def _fast_in_maps(hidden_states, attention_mask, Wq, Wk, Wv, Wo):
    cos, sin = _rope_tables()
    cos_bf = cos.astype(bfloat16)
    sin_bf = sin.astype(bfloat16)

    xt = [np.ascontiguousarray(hidden_states[b].T).astype(bfloat16)
          for b in range(B)]
    wqt = np.ascontiguousarray(Wq.T).astype(bfloat16)
    wkt = np.ascontiguousarray(Wk.T).astype(bfloat16)
    wvt = np.ascontiguousarray(Wv.T).astype(bfloat16)
    wot = np.ascontiguousarray(Wo.T).astype(bfloat16)
    mask = np.asarray(attention_mask, dtype=np.float32).reshape(S, S)

    # wkvp: slabs [K_g0, V_g0, K_g1, V_g1, K_g2, V_g2], each [128, 18*HD]
    wkvp = np.concatenate([
        _pack_cols(m, [(g * HD, HD)]).reshape(-1)
        for g in range(NKV) for m in (wkt, wvt)])
    # wqp: per q head hh, [128, 18*HD]
    wqp = np.concatenate([_pack_cols(wqt, [(hh * HD, HD)]).reshape(-1)
                          for hh in range(NH)])
    # wop: nine 256-col output blocks, each [128, 18*256]
    wop = np.concatenate([_pack_cols(wot, [(u * HD, HD)]).reshape(-1)
                          for u in range(H // HD)])

    in_maps = []
    for c in range(NCORES):
        b, w = c // 4, c % 4
        blocks = _qblocks(w)
        qrows = np.concatenate([np.arange(bl * P, (bl + 1) * P)
                                for bl in blocks])
        chunk = slice(w * CHUNK, (w + 1) * CHUNK)
        # maskp[p, t*P+qi] = 16*mask[qrow(slot_t, qi), t*P+p]
        maskp = np.empty((P, NKT * P), dtype=np.float32)
        for t in range(NKT):
            sl = 3 - t // 4           # slot masked at this k-tile
            bl = blocks[sl]
            maskp[:, t * P:(t + 1) * P] = \
                16.0 * mask[bl * P:(bl + 1) * P, t * P:(t + 1) * P].T
        in_maps.append({
            "xkv": _pack_act(xt[b][:, chunk]),
            "xq": _pack_act(xt[b][:, qrows]),
            "wqp": wqp, "wkvp": wkvp, "wop": wop,
            "cosk": np.ascontiguousarray(cos_bf[:, chunk]),
            "sink": np.ascontiguousarray(sin_bf[:, chunk]),
            "cosq": np.ascontiguousarray(cos_bf[:, qrows]),
            "sinq": np.ascontiguousarray(sin_bf[:, qrows]),
            "maskp": maskp.astype(bfloat16),
        })
    return in_maps


def _fast_kernel(hidden_states, attention_mask, Wq, Wk, Wv, Wo):
    from concourse.bass_utils import run_bass_kernel_spmd

    if "nc_fast" not in _CACHE:
        _CACHE["nc_fast"] = _build_nc_fast()
    nc = _CACHE["nc_fast"]
    in_maps = _fast_in_maps(hidden_states, attention_mask, Wq, Wk, Wv, Wo)
    res = run_bass_kernel_spmd(nc, in_maps, list(range(NCORES)))
    out = np.empty((B, S, H), dtype=np.float32)
    for c in range(NCORES):
        b, w = c // 4, c % 4
        r = np.asarray(res.results[c]["out"], dtype=np.float32)
        for j, bl in enumerate(_qblocks(w)):
            out[b, bl * P:(bl + 1) * P, :] = r[j * P:(j + 1) * P, :]
    return out


# ---------------------------------------------------------------------------
# dense fallback (arbitrary additive mask)
# ---------------------------------------------------------------------------

def _build_nc_dense():
    import concourse.bass as bass
    import concourse.tile as tile
    from concourse import bacc, mybir

    BF = mybir.dt.bfloat16
    F32 = mybir.dt.float32

    nc = bacc.Bacc(None, target_bir_lowering=False, debug=False,
                   num_devices=NCORES)

    d_xt = nc.dram_tensor("xt", [H, S], BF, kind="ExternalInput").ap()
    d_xq = nc.dram_tensor("xq", [H, SQ], BF, kind="ExternalInput").ap()
    d_wqt = nc.dram_tensor("wqt", [H, H], BF, kind="ExternalInput").ap()
    d_wkt = nc.dram_tensor("wkt", [H, NKV * HD], BF, kind="ExternalInput").ap()
    d_wvt = nc.dram_tensor("wvt", [H, NKV * HD], BF, kind="ExternalInput").ap()
    d_wot = nc.dram_tensor("wot", [H, H], BF, kind="ExternalInput").ap()
    d_cosk = nc.dram_tensor("cosk", [P, S], BF, kind="ExternalInput").ap()
    d_sink = nc.dram_tensor("sink", [P, S], BF, kind="ExternalInput").ap()
    d_cosq = nc.dram_tensor("cosq", [P, SQ], BF, kind="ExternalInput").ap()
    d_sinq = nc.dram_tensor("sinq", [P, SQ], BF, kind="ExternalInput").ap()
    d_maskt = nc.dram_tensor("maskt", [S, SQ], BF, kind="ExternalInput").ap()
    d_out = nc.dram_tensor("out", [SQ, H], BF, kind="ExternalOutput").ap()

    NSEQ = S // P        # 16 key tiles of 128

    with tile.TileContext(nc) as tc:
        with (
            tc.tile_pool(name="res", bufs=1) as res,
            tc.tile_pool(name="xtk", bufs=6) as xtk_pool,
            tc.tile_pool(name="xtv", bufs=6) as xtv_pool,
            tc.tile_pool(name="wq", bufs=6) as wq_pool,
            tc.tile_pool(name="wk", bufs=4) as wk_pool,
            tc.tile_pool(name="wv", bufs=4) as wv_pool,
            tc.tile_pool(name="wo", bufs=6) as wo_pool,
            tc.tile_pool(name="rtmp", bufs=6) as rtmp_pool,
            tc.tile_pool(name="expin", bufs=4) as expin_pool,
            tc.tile_pool(name="expt", bufs=6) as expt_pool,
            tc.tile_pool(name="recip", bufs=3) as recip_pool,
            tc.tile_pool(name="osb", bufs=4) as osb_pool,
            tc.tile_pool(name="ps", bufs=8, space="PSUM") as ps_pool,
        ):
            # ---- resident tiles ----
            ones_sb = res.tile([P, P], BF, tag="ones")
            nc.vector.memset(ones_sb[:], 1.0)

            xq_sb = res.tile([P, NHC * SQ], BF, tag="xq")
            cosq_sb = res.tile([P, SQ], BF, tag="cosq")
            sinq_sb = res.tile([P, SQ], BF, tag="sinq")
            cosk_sb = res.tile([P, S], BF, tag="cosk")
            nc.sync.dma_start(cosk_sb[:], d_cosk[:])
            sink_sb = res.tile([P, S], BF, tag="sink")
            nc.sync.dma_start(sink_sb[:], d_sink[:])
            maskt_sb = res.tile([P, NSEQ * SQ], BF, tag="maskt")

            qt_sb = res.tile([P, NHC * SQ], BF, tag="qt")     # rope'd Q^T
            kt_sb = res.tile([P, 2 * NKV * S], BF, tag="kt")  # rope'd K^T
            v_sb = res.tile([P, NSEQ * DK], BF, tag="v")      # V seq-major
            avt_sb = res.tile([P, NHC * SQ], BF, tag="avt")   # AV^T

            def rope_pair(top_ps, bot_ps, cos_sb, sin_sb, cs, width,
                          out_ap_top, out_ap_bot):
                ta = rtmp_pool.tile([P, SQ], F32, tag="rt")
                nc.vector.tensor_mul(ta[:, :width], top_ps, cos_sb[:, cs:cs + width])
                tb = rtmp_pool.tile([P, SQ], F32, tag="rt")
                nc.vector.tensor_mul(tb[:, :width], bot_ps, sin_sb[:, cs:cs + width])
                nc.vector.tensor_sub(out_ap_top, ta[:, :width], tb[:, :width])
                tc_ = rtmp_pool.tile([P, SQ], F32, tag="rt")
                nc.vector.tensor_mul(tc_[:, :width], bot_ps, cos_sb[:, cs:cs + width])
                td = rtmp_pool.tile([P, SQ], F32, tag="rt")
                nc.vector.tensor_mul(td[:, :width], top_ps, sin_sb[:, cs:cs + width])
                nc.vector.tensor_add(out_ap_bot, tc_[:, :width], td[:, :width])

            # ---- K projection + RoPE:  K^T[dk, s] = Wk @ X^T ----
            for n in range(S // SQ):            # 4 seq chunks of 512
                accs = [ps_pool.tile([P, SQ], F32, tag="ps", name="kacc") for _ in range(6)]
                for h in range(NHC):
                    xt_t = xtk_pool.tile([P, SQ], BF, tag="xtk")
                    nc.sync.dma_start(xt_t[:],
                                      d_xt[h * P:(h + 1) * P,
                                           n * SQ:(n + 1) * SQ])
                    wt = wk_pool.tile([P, DK], BF, tag="wk")
                    nc.sync.dma_start(wt[:], d_wkt[h * P:(h + 1) * P, :])
                    for m in range(6):
                        nc.tensor.matmul(accs[m][:], wt[:, m * P:(m + 1) * P],
                                         xt_t[:],
                                         start=(h == 0), stop=(h == NHC - 1))
                for g in range(NKV):
                    base0 = (2 * g) * S + n * SQ
                    base1 = (2 * g + 1) * S + n * SQ
                    rope_pair(accs[2 * g][:], accs[2 * g + 1][:],
                              cosk_sb, sink_sb, n * SQ, SQ,
                              kt_sb[:, base0:base0 + SQ],
                              kt_sb[:, base1:base1 + SQ])

            # ---- V projection (seq-major):  V[s, dv] = X^T.T @ Wv^T ----
            for sg in range(NSEQ // 2):         # groups of 2 seq-chunks
                accs = []
                for j in range(2):
                    accs.append((ps_pool.tile([P, SQ], F32, tag="ps", name="vacc0"),
                                 ps_pool.tile([P, SQ], F32, tag="ps", name="vacc1")))
                for h in range(NHC):
                    xt_t = xtv_pool.tile([P, 2 * P], BF, tag="xtv")
                    nc.sync.dma_start(xt_t[:],
                                      d_xt[h * P:(h + 1) * P,
                                           sg * 2 * P:sg * 2 * P + 2 * P])
                    wt = wv_pool.tile([P, DK], BF, tag="wv")
                    nc.sync.dma_start(wt[:], d_wvt[h * P:(h + 1) * P, :])
                    for j in range(2):
                        nc.tensor.matmul(accs[j][0][:],
                                         xt_t[:, j * P:(j + 1) * P],
                                         wt[:, :SQ],
                                         start=(h == 0), stop=(h == NHC - 1))
                        nc.tensor.matmul(accs[j][1][:, :DK - SQ],
                                         xt_t[:, j * P:(j + 1) * P],
                                         wt[:, SQ:DK],
                                         start=(h == 0), stop=(h == NHC - 1))
                for j in range(2):
                    s_idx = sg * 2 + j
                    nc.vector.tensor_copy(
                        v_sb[:, s_idx * DK:s_idx * DK + SQ], accs[j][0][:])
                    nc.vector.tensor_copy(
                        v_sb[:, s_idx * DK + SQ:(s_idx + 1) * DK],
                        accs[j][1][:, :DK - SQ])

            # ---- Q projection + RoPE:  Q^T[dq, q] = Wq @ X_q^T ----
            for h in range(NHC):
                nc.sync.dma_start(xq_sb[:, h * SQ:(h + 1) * SQ],
                                  d_xq[h * P:(h + 1) * P, :])
            nc.sync.dma_start(cosq_sb[:], d_cosq[:])
            nc.sync.dma_start(sinq_sb[:], d_sinq[:])
            for gi, heads in enumerate(([0, 1], [2, 3], [4, 5],
                                        [6, 7], [8])):
                mchunks = [2 * hh + half for hh in heads for half in range(2)]
                accs = {}
                for m in mchunks:
                    accs[m] = ps_pool.tile([P, SQ], F32, tag="ps", name="qacc")
                for h in range(NHC):
                    wt = wq_pool.tile([P, P * 4], BF, tag="wq")
                    w = P * len(mchunks)
                    nc.sync.dma_start(
                        wt[:, :w],
                        d_wqt[h * P:(h + 1) * P,
                              mchunks[0] * P:mchunks[0] * P + w])
                    for j, m in enumerate(mchunks):
                        nc.tensor.matmul(
                            accs[m][:], wt[:, j * P:(j + 1) * P],
                            xq_sb[:, h * SQ:(h + 1) * SQ],
                            start=(h == 0), stop=(h == NHC - 1))
                for hh in heads:
                    rope_pair(accs[2 * hh][:], accs[2 * hh + 1][:],
                              cosq_sb, sinq_sb, 0, SQ,
                              qt_sb[:, (2 * hh) * SQ:(2 * hh + 1) * SQ],
                              qt_sb[:, (2 * hh + 1) * SQ:(2 * hh + 2) * SQ])

            # ---- attention per q-head ----
            for k in range(NSEQ):
                nc.sync.dma_start(maskt_sb[:, k * SQ:(k + 1) * SQ],
                                  d_maskt[k * P:(k + 1) * P, :])
            inv_sqrt_hd = 1.0 / float(np.sqrt(HD))
            from concourse.mybir import AluOpType, ActivationFunctionType
            for hh in range(NH):
                g = hh // GROUPS
                qtop = qt_sb[:, (2 * hh) * SQ:(2 * hh + 1) * SQ]
                qbot = qt_sb[:, (2 * hh + 1) * SQ:(2 * hh + 2) * SQ]
                sum_ps = ps_pool.tile([P, SQ], F32, tag="ps")
                av_ps = [ps_pool.tile([P, SQ], F32, tag="ps", name="avps") for _ in range(2)]
                for k in range(NSEQ):
                    s_ps = ps_pool.tile([P, SQ], F32, tag="ps")
                    nc.tensor.matmul(
                        s_ps[:],
                        kt_sb[:, (2 * g) * S + k * P:(2 * g) * S + (k + 1) * P],
                        qtop, start=True, stop=False)
                    nc.tensor.matmul(
                        s_ps[:],
                        kt_sb[:, (2 * g + 1) * S + k * P:(2 * g + 1) * S + (k + 1) * P],
                        qbot, start=False, stop=True)
                    e_in = expin_pool.tile([P, SQ], F32, tag="ei")
                    nc.vector.scalar_tensor_tensor(
                        e_in[:], s_ps[:], inv_sqrt_hd,
                        maskt_sb[:, k * SQ:(k + 1) * SQ],
                        op0=AluOpType.mult, op1=AluOpType.add)
                    e_t = expt_pool.tile([P, SQ], BF, tag="et")
                    nc.scalar.activation(e_t[:], e_in[:],
                                         ActivationFunctionType.Exp)
                    nc.tensor.matmul(sum_ps[:], ones_sb[:], e_t[:],
                                     start=(k == 0), stop=(k == NSEQ - 1))
                    for m in range(2):
                        nc.tensor.matmul(
                            av_ps[m][:],
                            v_sb[:, k * DK + g * HD + m * P:
                                 k * DK + g * HD + (m + 1) * P],
                            e_t[:], start=(k == 0), stop=(k == NSEQ - 1))
                rec = recip_pool.tile([P, SQ], F32, tag="rc")
                nc.vector.reciprocal(rec[:], sum_ps[:])
                for m in range(2):
                    nc.vector.tensor_mul(
                        avt_sb[:, (2 * hh + m) * SQ:(2 * hh + m + 1) * SQ],
                        av_ps[m][:], rec[:])

            # ---- output projection: out[q, o] = AV^T.T @ Wo^T ----
            for og, ow in ((0, 512), (512, 512), (1024, 512), (1536, 512),
                           (2048, 256)):
                accs = [ps_pool.tile([P, SQ], F32, tag="ps", name="oacc") for _ in range(NQ)]
                for c in range(NHC):
                    wt = wo_pool.tile([P, SQ], BF, tag="wo")
                    nc.sync.dma_start(wt[:, :ow],
                                      d_wot[c * P:(c + 1) * P, og:og + ow])
                    for m in range(NQ):
                        nc.tensor.matmul(
                            accs[m][:, :ow],
                            avt_sb[:, c * SQ + m * P:c * SQ + (m + 1) * P],
                            wt[:, :ow],
                            start=(c == 0), stop=(c == NHC - 1))
                for m in range(NQ):
                    o_sb = osb_pool.tile([P, SQ], F32, tag="ob")
                    nc.vector.tensor_copy(o_sb[:, :ow], accs[m][:, :ow])
                    nc.sync.dma_start(d_out[m * P:(m + 1) * P, og:og + ow],
                                      o_sb[:, :ow])

    nc.compile()
    return nc


def _dense_kernel(hidden_states, attention_mask, Wq, Wk, Wv, Wo):
    from concourse.bass_utils import run_bass_kernel_spmd

    if "nc_dense" not in _CACHE:
        _CACHE["nc_dense"] = _build_nc_dense()
    nc = _CACHE["nc_dense"]
    cos, sin = _rope_tables()
    cos_bf = cos.astype(bfloat16)
    sin_bf = sin.astype(bfloat16)

    xt = [np.ascontiguousarray(hidden_states[b].T).astype(bfloat16)
          for b in range(B)]
    wqt = np.ascontiguousarray(Wq.T).astype(bfloat16)
    wkt = np.ascontiguousarray(Wk.T).astype(bfloat16)
    wvt = np.ascontiguousarray(Wv.T).astype(bfloat16)
    wot = np.ascontiguousarray(Wo.T).astype(bfloat16)
    mask = np.asarray(attention_mask, dtype=np.float32).reshape(S, S)

    in_maps = []
    for c in range(NCORES):
        b, w = c // 4, c % 4
        rows = slice(w * SQ, (w + 1) * SQ)
        in_maps.append({
            "xt": xt[b],
            "xq": np.ascontiguousarray(xt[b][:, rows]),
            "wqt": wqt, "wkt": wkt, "wvt": wvt, "wot": wot,
            "cosk": cos_bf, "sink": sin_bf,
            "cosq": np.ascontiguousarray(cos_bf[:, rows]),
            "sinq": np.ascontiguousarray(sin_bf[:, rows]),
            "maskt": np.ascontiguousarray(mask[rows, :].T).astype(bfloat16),
        })

    res = run_bass_kernel_spmd(nc, in_maps, list(range(NCORES)))
    out = np.empty((B, S, H), dtype=np.float32)
    for c in range(NCORES):
        b, w = c // 4, c % 4
        out[b, w * SQ:(w + 1) * SQ, :] = res.results[c]["out"]
    return out


def kernel(hidden_states, attention_mask, Wq, Wk, Wv, Wo):
    mask = np.asarray(attention_mask, dtype=np.float32).reshape(S, S)
    if _is_causal(mask):
        return _fast_kernel(hidden_states, attention_mask, Wq, Wk, Wv, Wo)
    return _dense_kernel(hidden_states, attention_mask, Wq, Wk, Wv, Wo)
